# revision 1
# baseline (speedup 1.0000x reference)
"""CPAMDec attention-decoder kernel for 8 Trainium2 NeuronCores.

Reference computation (per batch n of N=8):
    q  = x_n^T @ wq^T + bq          (HW=4096, C4=128)
    k  = y_n @ wk^T + bk            (K=32, C4=128)
    v  = y_n @ wv^T + bv            (K=32, C=512)
    attn = softmax(q @ k^T, axis=-1)        (HW, K)
    out = scale * (v^T @ attn^T) + x_n      (C, HW)

Sharding: pure data parallel — core i computes batch i. Params are
replicated (host pre-transposes them so no on-device transposes are
needed). Heavy matmuls run in float32r (1 cycle/row at N=512); the
residual add reads the original fp32 bits of x, so the dominant output
term is exact.

Structure: column-streaming pipeline. x arrives in 8 chunks of 512
pixels (one strided DMA each); each chunk runs the full chain
q -> energy -> exp -> sum -> 1/sum -> attn -> out-matmul -> +bv+x -> out
so input DMA, compute on all engines, and output DMA overlap. PSUM is
partitioned per stage (q:2, e:2, s:2, o:2 banks) for cross-chunk
double buffering.

Bias folding:
  - bq contributes a per-key bias e_b[j] = sum_o bq[o]*k[j,o], applied
    inside the exp() activation (exact algebra).
  - bv enters as a per-partition scalar in the fused output op
    osb = (o_ps + s*bv[c]) + x, using sum_j attn[p,j] = 1.
"""

import sys

sys.path.insert(0, "/opt/trn_rl_repo")

import numpy as np

import concourse.bacc as bacc
import concourse.mybir as mybir
import concourse.tile as tile
from concourse.alu_op_type import AluOpType
from concourse.bass_utils import run_bass_kernel_spmd

F32 = mybir.dt.float32
F32R = mybir.dt.float32r
AF = mybir.ActivationFunctionType

N, C, H, W, K = 8, 512, 64, 64, 32
HW = H * W            # 4096
C4 = C // 4           # 128
PC = 512              # free-dim chunk (1 PSUM bank of fp32)
NPC = HW // PC        # 8 chunks
KC = C // 128         # 4 contraction chunks
CT = C // 128         # 4 output row-tiles


def _load_consts(nc, tc, cst, cdma):
    """Constant loads on the sync HWDGE ring (wq first — q(0) needs it)."""
    yt, wq, wkt, wv = [], [], [], []
    for k in range(KC):
        t = cst.tile([128, C4], F32R, name=f"wq{k}", tag=f"wq{k}")
        cdma.dma_start(t[:], nc.t.wqT[k * 128:(k + 1) * 128, :].bitcast(F32R))
        wq.append(t)
    for k in range(KC):
        t = cst.tile([128, 4 * K], F32R, name=f"yt{k}", tag=f"yt{k}")
        cdma.dma_start(t[:], nc.t.yT[k * 128:(k + 1) * 128, :].bitcast(F32R))
        yt.append(t)
    for k in range(KC):
        t = cst.tile([128, C4], F32R, name=f"wk{k}", tag=f"wk{k}")
        cdma.dma_start(t[:], nc.t.wkT[k * 128:(k + 1) * 128, :].bitcast(F32R))
        wkt.append(t)

    wv = []
    for k in range(KC):
        t = cst.tile([128, C], F32R, name=f"wv{k}", tag=f"wv{k}")
        cdma.dma_start(t[:], nc.t.wvT[k * 128:(k + 1) * 128, :].bitcast(F32R))
        wv.append(t)
    bq_r = cst.tile([C4, K], F32R, name="bq_r", tag="bq_r")
    cdma.dma_start(bq_r[:], nc.t.bqb[:].bitcast(F32R))
    bk_sb = cst.tile([C4, 1], F32, name="bk_sb", tag="bk_sb")
    cdma.dma_start(bk_sb[:], nc.t.bk[:])
    bvt_sb = cst.tile([C4, CT], F32, name="bvt_sb", tag="bvt_sb")
    cdma.dma_start(bvt_sb[:], nc.t.bvt[:])
    s_bc32 = cst.tile([K, 1], F32, name="s_bc32", tag="s_bc32")
    nc.gpsimd.dma_start(
        s_bc32[:], nc.t.s[:].partition_broadcast(K).squeeze(-1))
    s_bc128 = cst.tile([C4, 1], F32, name="s_bc128", tag="s_bc128")
    nc.gpsimd.dma_start(
        s_bc128[:], nc.t.s[:].partition_broadcast(C4).squeeze(-1))
    ones32 = cst.tile([K, 128], F32R, name="ones32", tag="ones32")
    nc.gpsimd.dma_start(
        ones32[:],
        nc.t.ones[0:1, 0:128].bitcast(F32R).partition_broadcast(K).squeeze(1))
    return yt, wq, wkt, wv, bq_r, bk_sb, bvt_sb, s_bc32, s_bc128, ones32


def _emit(nc, tc):
    sync = nc.sync
    cdma = sync  # constants share the sync HWDGE ring (idle early)

    with (
        tc.tile_pool(name="const", bufs=1) as cst,
        tc.tile_pool(name="xbuf", bufs=1) as xp,
        tc.tile_pool(name="work", bufs=3) as wk_pool,
        tc.tile_pool(name="ps", bufs=2, space="PSUM") as ps,
    ):
        # x column chunks: (128 part, 4 c-tiles, 512 cols) strided loads
        xs = [None] * NPC

        def load_chunk(pc):
            t = xp.tile([128, KC, PC], F32R, name=f"xs{pc}", tag=f"xs{pc}")
            src = nc.t.x[:, pc * PC:(pc + 1) * PC].bitcast(F32R).rearrange(
                "(k p) f -> p k f", p=128)
            nc.scalar.dma_start(t[:], src)
            xs[pc] = t

        # prefetch the first two chunks before anything else hits the ring
        load_chunk(0)
        load_chunk(1)

        # PE warm-up: the HAM clock gate only unthrottles (1.2 -> 2.4 GHz)
        # after ~3.4us of *sustained* matmul activity, and it re-throttles
        # only after ~3.4us of continuous idle. The PE sits idle until x
        # arrives (~13us) — burn that window with dummy matmuls so every
        # real matmul afterwards runs at full clock.
        dmy_w = cst.tile([128, 128], F32R, name="dmy_w", tag="dmy_w")
        sync.dma_start(dmy_w[:], nc.t.x[0:128, 0:128].bitcast(F32R))
        dmy_x = cst.tile([128, PC], F32R, name="dmy_x", tag="dmy_x")
        sync.dma_start(dmy_x[:], nc.t.x[0:128, 0:PC].bitcast(F32R))
        dmy_ps = ps.tile([128, PC], F32, name="dmy_ps", tag="q", bufs=2)
        for _ in range(18):
            nc.tensor.matmul(dmy_ps[:], dmy_w[:], dmy_x[:],
                             start=True, stop=True)

        (yt, wq, wkt, wv, bq_r, bk_sb, bvt_sb, s_bc32, s_bc128,
         ones32) = _load_consts(nc, tc, cst, cdma)

        pro = {}

        def emit_prologue():
            # kT (with bk), v (scaled by s), e_b, s*bv — emitted after
            # stage_q(0) so q(0) leads the PE queue.
            kt_ps = ps.tile([C4, 4 * K], F32, name="kt_ps", tag="e", bufs=1)
            for k in range(KC):
                nc.tensor.matmul(kt_ps[:], wkt[k][:], yt[k][:],
                                 start=(k == 0), stop=(k == KC - 1))
            ktb4 = cst.tile([C4, 4 * K], F32R, name="ktb4", tag="ktb4")
            nc.scalar.activation(out=ktb4[:], in_=kt_ps[:], func=AF.Identity,
                                 bias=bk_sb[:], scale=1.0)

            v_ps = ps.tile([K, C], F32, name="v_ps", tag="s", bufs=1)
            for k in range(KC):
                nc.tensor.matmul(v_ps[:], yt[k][:, 0:K], wv[k][:],
                                 start=(k == 0), stop=(k == KC - 1))
            v_sb = cst.tile([K, C], F32R, name="v_sb", tag="v_sb")
            nc.scalar.activation(out=v_sb[:], in_=v_ps[:], func=AF.Copy,
                                 bias=0.0, scale=s_bc32[:])
            # partition-stacked copy for row-packed final matmuls:
            # vstack[32*ct + j, m] = v_sb[j, 128*ct + m]
            vstack = cst.tile([128, 128], F32R, name="vstack", tag="vstack")
            for ct in range(CT):
                nc.gpsimd.dma_start(
                    vstack[32 * ct:32 * (ct + 1), :],
                    v_sb[:, 128 * ct:128 * (ct + 1)])

            eb_ps = ps.tile([4 * K, K], F32, name="eb_ps", tag="o", bufs=4)
            nc.tensor.matmul(eb_ps[:], ktb4[:], bq_r[:], start=True,
                             stop=True)
            e_b4 = cst.tile([4 * K, 1], F32, name="e_b4", tag="e_b4")
            nc.scalar.activation(out=e_b4[:], in_=eb_ps[:, 0:1],
                                 func=AF.Copy, scale=1.0)

            bvs = cst.tile([C4, CT], F32, name="bvs", tag="bvs")
            nc.vector.tensor_scalar_mul(bvs[:], bvt_sb[:], s_bc128[:])
            pro.update(ktb4=ktb4, v_sb=v_sb, vstack=vstack, e_b4=e_b4,
                       bvs=bvs)

        # ------------- software-pipelined main loop over column chunks ----
        # Stages are skewed so every PE instruction's inputs were produced
        # in an earlier iteration (the engine queues are in-order; without
        # the skew the PE stalls mid-chain waiting on ACT/DVE/GpSimd).
        #   step:   q(step)   e/exp(step-1)   sum/rec/mul(step-2)
        #           finals/add/store(step-3)
        qtcs = [None] * NPC
        expts = [None] * NPC
        attns = [None] * NPC

        def stage_q(pc):
            xt = xs[pc]
            q_ps = ps.tile([C4, PC], F32, name=f"q_ps{pc}", tag="q", bufs=2)
            for k in range(KC):
                nc.tensor.matmul(q_ps[:], wq[k][:], xt[:, k, :],
                                 start=(k == 0), stop=(k == KC - 1))
            qtc = wk_pool.tile([C4, PC], F32R, name="qtc", tag="qtc", bufs=4)
            nc.scalar.activation(out=qtc[:], in_=q_ps[:], func=AF.Copy,
                                 scale=1.0)
            qtcs[pc] = qtc

        def stage_energy(pc):
            e_ps = ps.tile([128, PC], F32, name=f"e_ps{pc}", tag="e", bufs=1)
            nc.tensor.matmul(e_ps[:], pro['ktb4'][:], qtcs[pc][:],
                             start=True, stop=True)
            expt = wk_pool.tile([128, PC], F32R, name="expt", tag="expt",
                                bufs=4)
            nc.scalar.activation(out=expt[:], in_=e_ps[:], func=AF.Exp,
                                 bias=pro['e_b4'][:], scale=1.0)
            expts[pc] = expt

        def stage_softmax(pc):
            s_ps = ps.tile([128, PC], F32, name=f"s_ps{pc}", tag="s", bufs=1)
            nc.tensor.matmul(s_ps[:], ones32[:], expts[pc][0:K, :],
                             start=True, stop=True)
            rec = wk_pool.tile([128, PC], F32, name="rec", tag="rec", bufs=4)
            nc.vector.reciprocal_approx_fast(
                out=rec[:], in_=s_ps[:].bitcast(F32))
            attn = wk_pool.tile([128, PC], F32R, name="attn", tag="attn",
                                bufs=4)
            nc.vector.tensor_mul(attn[:], expts[pc][:].bitcast(F32), rec[:])
            attns[pc] = attn

        def stage_out(pc):
            sl = slice(pc * PC, (pc + 1) * PC)
            xt = xs[pc]
            attn = attns[pc]
            osb = wk_pool.tile([128, CT, PC], F32, name="osb", tag="osb",
                               bufs=3)
            for ct in range(CT):
                o_ps = ps.tile([128, PC], F32, name=f"o_ps{pc}_{ct}",
                               tag="o", bufs=4)
                nc.tensor.matmul(o_ps[:],
                                 pro['vstack'][32 * ct:32 * (ct + 1), :],
                                 attn[32 * ct:32 * (ct + 1), :],
                                 start=True, stop=True,
                                 tile_position=(32 * ct, 0))
                nc.vector.scalar_tensor_tensor(
                    out=osb[:, ct, :], in0=o_ps[:],
                    scalar=pro['bvs'][:, ct:ct + 1],
                    in1=xt[:, ct, :].bitcast(F32),
                    op0=AluOpType.add, op1=AluOpType.add)
            dst = nc.t.out[:, sl].rearrange("(k p) f -> p k f", p=128)
            sync.dma_start(dst, osb[:])

        stage_q(0)
        emit_prologue()
        for step in range(1, NPC + 3):
            if 2 <= step + 3 < NPC:
                load_chunk(step + 3)
            if step == 1:
                load_chunk(2)
                load_chunk(3)
            if step < NPC:
                stage_q(step)
            if 0 <= step - 1 < NPC:
                stage_energy(step - 1)
            if 0 <= step - 2 < NPC:
                stage_softmax(step - 2)
            if 0 <= step - 3 < NPC:
                stage_out(step - 3)


class _T:
    """Attribute access to declared dram params."""
    def __init__(self):
        self.__dict__ = {}


_NC_CACHE = []


def _build():
    if _NC_CACHE:
        return _NC_CACHE[0]
    nc = bacc.Bacc(target_bir_lowering=False)
    nc.t = _T()
    t = nc.t
    t.x = nc.declare_dram_parameter("x", [C, HW], F32, isOutput=False)
    t.yT = nc.declare_dram_parameter("yT", [C, 4 * K], F32, isOutput=False)
    t.wqT = nc.declare_dram_parameter("wqT", [C, C4], F32, isOutput=False)
    t.wkT = nc.declare_dram_parameter("wkT", [C, C4], F32, isOutput=False)
    t.wvT = nc.declare_dram_parameter("wvT", [C, C], F32, isOutput=False)
    t.bqb = nc.declare_dram_parameter("bqb", [C4, K], F32, isOutput=False)
    t.bk = nc.declare_dram_parameter("bk", [C4, 1], F32, isOutput=False)
    t.bvt = nc.declare_dram_parameter("bvt", [C4, CT], F32, isOutput=False)
    t.s = nc.declare_dram_parameter("s", [1, 1], F32, isOutput=False)
    t.ones = nc.declare_dram_parameter("ones", [1, HW], F32, isOutput=False)
    t.out = nc.declare_dram_parameter("out", [C, HW], F32, isOutput=True)
    with tile.TileContext(nc) as tc:
        _emit(nc, tc)
    nc.finalize()
    _NC_CACHE.append(nc)
    return nc


def _in_maps(x, y, wq, bq, wk, bk, wv, bv, scale):
    x = np.ascontiguousarray(x, dtype=np.float32).reshape(N, C, HW)
    yT = np.ascontiguousarray(
        np.tile(np.transpose(y, (0, 2, 1)), (1, 1, 4)), dtype=np.float32)
    wqT = np.ascontiguousarray(wq.T, dtype=np.float32)
    wkT = np.ascontiguousarray(wk.T, dtype=np.float32)
    wvT = np.ascontiguousarray(wv.T, dtype=np.float32)
    bqb = np.ascontiguousarray(
        np.broadcast_to(np.float32(bq).reshape(C4, 1), (C4, K)),
        dtype=np.float32)
    bk = np.ascontiguousarray(bk, dtype=np.float32).reshape(C4, 1)
    bvt = np.ascontiguousarray(
        np.float32(bv).reshape(CT, C4).T, dtype=np.float32)
    s = np.ascontiguousarray(scale, dtype=np.float32).reshape(1, 1)
    return [
        {
            "x": x[i], "yT": yT[i], "wqT": wqT, "wkT": wkT, "wvT": wvT,
            "bqb": bqb, "bk": bk, "bvt": bvt, "s": s,
            "ones": np.ones((1, HW), dtype=np.float32),
        }
        for i in range(N)
    ]


def _run(inputs, **kwargs):
    nc = _build()
    return run_bass_kernel_spmd(nc, _in_maps(**inputs),
                                core_ids=list(range(N)), **kwargs)


def kernel(**inputs) -> np.ndarray:
    res = _run(inputs)
    out = np.stack([res.results[i]["out"] for i in range(N)])
    return out.reshape(N, C, H, W).astype(np.float32)

